# revision 1
# baseline (speedup 1.0000x reference)
"""Trainium2 Bass kernel for nn_ArgumentClassification (2-layer BiLSTM tagger).

Sharding: data-parallel over batch B=32 across 8 NeuronCores (4 rows each),
LSTM/Linear weights replicated. No collectives.

Per-core pipeline (all compute on device):
  1. mean over 4 transformer layers of hidden_states  -> x [4,256,768]
  2. predicate-relative delta + role mask features    -> x_ext [4,256,770]
     (x is built directly in transposed layout x.T [770, S*B] via PE transposes)
  3. L0 BiLSTM: input projection (batched matmul over all timesteps, biases
     folded in via a ones-row), then the sequential 256-step scan in
     gates-transposed layout [2048, B] with Whh stationary on the PE.
     Forward/backward directions interleaved, and gate groups emitted in
     (g,i,f,o) order into separate PSUM banks so the DVE/ACT nonlinearity
     chain overlaps the remaining matmuls of the step.
  4. L1 BiLSTM: same, input = [h0f; h0b].
  5. out = h1 @ W_out.T + b_out, PE-transposed back to [B,S,30] and DMA'd out.

Gate order is host-permuted from PyTorch's (i,f,g,o) to (i,f,o,g) so the scan
needs only two activation instructions per step: sigmoid over tiles 0:12 and
tanh over tiles 12:16.
"""
import sys

sys.path.insert(0, "/opt/trn_rl_repo")

import numpy as np
import ml_dtypes

import concourse.bass as bass
import concourse.tile as tile
from concourse import bacc, mybir
from concourse.bass import ds
from concourse.masks import make_identity

BF16 = mybir.dt.bfloat16
F32 = mybir.dt.float32
AF = mybir.ActivationFunctionType
OP = mybir.AluOpType

B, S, E, H, C = 32, 256, 768, 512, 30
NCORES = 8
BL = B // NCORES          # 4 rows per core
SB = S * BL               # 1024 columns, ordered (t, b): col = t*BL + b
G = 4 * H                 # 2048 gate rows
MT = G // 128             # 16 gate m-tiles
KH = H // 128             # 4 hidden k-tiles
K0 = 7                    # L0 input k-tiles ([770 + ones-row] padded to 896)
K1 = 9                    # L1 input k-tiles (1024 + ones-row -> 1152)
UNROLL = 256

_cache = {}


def _bf(a):
    return np.asarray(a, dtype=ml_dtypes.bfloat16)


def _prep_weights(inp):
    """Host-side: permute gates to (i,f,o,g), transpose, pad, fold biases,
    tile for SBUF. Returns dict of name -> np array matching dram params."""
    perm = np.concatenate([
        np.arange(0, H),          # i
        np.arange(H, 2 * H),      # f
        np.arange(3 * H, 4 * H),  # o
        np.arange(2 * H, 3 * H),  # g
    ])
    out = {}

    def tile_k(a, nk):
        # [nk*128, M] -> [128, nk, M]
        return np.ascontiguousarray(
            a.reshape(nk, 128, a.shape[1]).transpose(1, 0, 2))

    def tile_km(a, nk):
        # [nk*128, 16*128] -> [16, 128, nk, 128]  (per-m-block contiguous)
        m = a.shape[1] // 128
        return np.ascontiguousarray(
            a.reshape(nk, 128, m, 128).transpose(2, 1, 0, 3))

    for d in ("f", "b"):
        # layer 0
        wih = inp[f"Wih_l0{d}"][perm]                     # [2048, 770]
        bias = (inp[f"bih_l0{d}"] + inp[f"bhh_l0{d}"])[perm]
        ext = np.zeros((K0 * 128, G), np.float32)
        ext[:768] = wih.T[:768]
        ext[768] = wih.T[768]      # delta coeffs at tile6 partition 0
        ext[800] = wih.T[769]      # mask coeffs at tile6 partition 32
        ext[832] = bias            # bias row at tile6 partition 64
        out[f"wih0{d}"] = _bf(tile_km(ext, K0))           # [16,128,7,128]
        whh = inp[f"Whh_l0{d}"][perm]                     # [2048, 512]
        out[f"whh0{d}"] = _bf(tile_k(whh.T, KH))          # [128, 4, 2048]
        # layer 1
        wih = inp[f"Wih_l1{d}"][perm]                     # [2048, 1024]
        bias = (inp[f"bih_l1{d}"] + inp[f"bhh_l1{d}"])[perm]
        ext = np.zeros((K1 * 128, G), np.float32)
        ext[:1024] = wih.T
        ext[1024] = bias
        out[f"wih1{d}"] = _bf(tile_km(ext, K1))           # [16,128,9,128]
        whh = inp[f"Whh_l1{d}"][perm]
        out[f"whh1{d}"] = _bf(tile_k(whh.T, KH))
    # output projection [1152, 30] with bias row at 1024
    ext = np.zeros((K1 * 128, C), np.float32)
    ext[:1024] = inp["W_out"].T
    ext[1024] = inp["b_out"]
    out["wout"] = _bf(tile_k(ext, K1))                    # [128, 9, 30]
    return out


def build_nc():
    nc = bacc.Bacc("TRN2", target_bir_lowering=False, debug=False,
                   num_devices=NCORES)
    hs = nc.dram_tensor("hs", [4, BL, S, E], BF16, kind="ExternalInput").ap()
    roles = nc.dram_tensor("roles", [1, SB], F32, kind="ExternalInput").ap()
    preds = nc.dram_tensor("preds", [1, SB], F32, kind="ExternalInput").ap()
    w = {}
    for d in ("f", "b"):
        w[f"wih0{d}"] = nc.dram_tensor(f"wih0{d}", [MT, 128, K0, 128], BF16,
                                       kind="ExternalInput").ap()
        w[f"wih1{d}"] = nc.dram_tensor(f"wih1{d}", [MT, 128, K1, 128], BF16,
                                       kind="ExternalInput").ap()
        w[f"whh0{d}"] = nc.dram_tensor(f"whh0{d}", [128, KH, G], BF16,
                                       kind="ExternalInput").ap()
        w[f"whh1{d}"] = nc.dram_tensor(f"whh1{d}", [128, KH, G], BF16,
                                       kind="ExternalInput").ap()
    w["wout"] = nc.dram_tensor("wout", [128, K1, C], BF16,
                               kind="ExternalInput").ap()
    out = nc.dram_tensor("out", [BL, S, C], F32, kind="ExternalOutput").ap()

    with tile.TileContext(nc) as tc:
        _emit(nc, tc, hs, roles, preds, w, out)
    nc.compile()
    return nc


def _emit(nc, tc, hs, roles, preds, w, out):
    from contextlib import ExitStack
    with ExitStack() as st:
        cpool = st.enter_context(tc.tile_pool(name="const", bufs=1))
        hlpool = st.enter_context(tc.tile_pool(name="hl", bufs=5))
        sumpool = st.enter_context(tc.tile_pool(name="sum", bufs=3))
        rpool = st.enter_context(tc.tile_pool(name="rows", bufs=1))
        xwpool = st.enter_context(tc.tile_pool(name="xw", bufs=2))
        scpool = st.enter_context(tc.tile_pool(name="sc", bufs=3))
        wpool = st.enter_context(tc.tile_pool(name="wts", bufs=2))
        pspool = st.enter_context(tc.tile_pool(name="ps", bufs=1, space="PSUM"))
        psg = st.enter_context(tc.tile_pool(name="psg", bufs=6, space="PSUM"))

        ident = cpool.tile([128, 128], F32, tag="ident")
        make_identity(nc, ident[:, :])
        ones_col = cpool.tile([128, 1], BF16, tag="onescol")
        nc.vector.memset(ones_col[:, :], 1.0)
        ones_row = cpool.tile([128, SB], BF16, tag="onesrow")
        nc.vector.memset(ones_row[:, :], 0.0)
        nc.vector.memset(ones_row[0:1, :], 1.0)

        # ---- x.T construction: [128, 7, SB] bf16 -------------------------
        xt = rpool.tile([128, K0, SB], BF16, tag="xt")
        hs_sbe = hs.rearrange("l b s e -> l s b e")
        for r in range(8):  # row-tiles of (t,b)
            acc = None
            hl = []
            for layer in range(4):
                t = hlpool.tile([128, E], BF16, tag="hl")
                nc.sync.dma_start(out=t[:, :],
                                  in_=hs_sbe[layer, 32 * r:32 * (r + 1), :, :])
                hl.append(t)
            s01 = sumpool.tile([128, E], F32, tag="sum")
            nc.vector.tensor_tensor(s01[:, :], hl[0][:, :], hl[1][:, :], OP.add)
            s23 = sumpool.tile([128, E], F32, tag="sum")
            nc.vector.tensor_tensor(s23[:, :], hl[2][:, :], hl[3][:, :], OP.add)
            ssum = sumpool.tile([128, E], F32, tag="sum")
            nc.vector.tensor_tensor(ssum[:, :], s01[:, :], s23[:, :], OP.add)
            for c in range(6):
                pt = pspool.tile([128, 128], F32, tag="tp")
                nc.tensor.transpose(pt[:, :], ssum[:, 128 * c:128 * (c + 1)],
                                    ident[:, :])
                nc.vector.tensor_scalar_mul(
                    xt[:, c, 128 * r:128 * (r + 1)], pt[:, :], 0.25)

        # ---- feature rows (delta, mask, ones) in xt[:, 6, :] -------------
        nc.vector.memset(xt[:, 6, :], 0.0)
        nc.vector.memset(xt[64:65, 6, :], 1.0)

        rrow = rpool.tile([1, SB], F32, tag="rrow")
        nc.sync.dma_start(out=rrow[:, :], in_=roles[:, :])
        prow = rpool.tile([1, SB], F32, tag="prow")
        nc.sync.dma_start(out=prow[:, :], in_=preds[:, :])
        m1 = rpool.tile([1, SB], F32, tag="m1")
        nc.vector.tensor_scalar(m1[:, :], rrow[:, :], 0.0, None, OP.not_equal)
        m2 = rpool.tile([1, SB], F32, tag="m2")
        nc.vector.tensor_scalar(m2[:, :], rrow[:, :], -100.0, None,
                                OP.not_equal)
        nc.vector.tensor_tensor(xt[32:33, 6, :], m1[:, :], m2[:, :], OP.mult)

        # mean_word row via ones-matmul over the 6 full e-tiles
        mw = rpool.tile([1, SB], F32, tag="mw")
        for ch in range(2):
            mp_ps = pspool.tile([1, 512], F32, tag="proj")
            for k in range(6):
                nc.tensor.matmul(mp_ps[:, :], ones_col[:, :],
                                 xt[:, k, 512 * ch:512 * (ch + 1)],
                                 start=(k == 0), stop=(k == 5))
            nc.vector.tensor_scalar_mul(mw[0:1, 512 * ch:512 * (ch + 1)],
                                        mp_ps[:, :], 1.0 / E)
        # first-predicate one-hot: oh = p * (cumsum(p) == 1)
        zrow = rpool.tile([1, SB], F32, tag="zrow")
        nc.vector.memset(zrow[:, :], 0.0)
        cs = rpool.tile([1, SB], F32, tag="cs")
        cs_b = cs.rearrange("p (t b) -> p b t", b=BL)
        pr_b = prow.rearrange("p (t b) -> p b t", b=BL)
        for b in range(BL):
            nc.vector.tensor_tensor_scan(cs_b[:, b, :], pr_b[:, b, :],
                                         zrow[0:1, 0:S], 0.0, OP.add, OP.add)
        oh = rpool.tile([1, SB], F32, tag="oh")
        nc.vector.tensor_scalar(oh[:, :], cs[:, :], 1.0, None, OP.is_equal)
        nc.vector.tensor_tensor(oh[:, :], oh[:, :], prow[:, :], OP.mult)
        nc.vector.tensor_tensor(oh[:, :], oh[:, :], mw[:, :], OP.mult)
        mpred = rpool.tile([1, BL], F32, tag="mpred")
        oh_b = oh.rearrange("p (t b) -> p b t", b=BL)
        nc.vector.tensor_reduce(mpred[:, :], oh_b[:, :, :],
                                mybir.AxisListType.X, OP.add)
        mw_b = mw.rearrange("p (t b) -> p b t", b=BL)
        xt6_b = xt.rearrange("p k (t b) -> p k b t", b=BL)
        for b in range(BL):
            nc.vector.tensor_scalar(xt6_b[0:1, 6, b, :], mw_b[:, b, :],
                                    mpred[0:1, b:b + 1], None, OP.subtract)

        # ---- projections + scans ----------------------------------------
        def projection(wih_dram, nk, rhs_of_k, xw):
            """xw[:, m, :] (bf16 [128, MT, SB]) = Wih_ext.T @ rhs (all t)."""
            for m in range(MT):
                wm = wpool.tile([128, nk, 128], BF16, tag="wihm")
                nc.sync.dma_start(out=wm[:, :, :], in_=wih_dram[m])
                for ch in range(2):
                    pp = pspool.tile([128, 512], F32, tag="proj")
                    for k in range(nk):
                        nc.tensor.matmul(pp[:, :], wm[:, k, :], rhs_of_k(k, ch),
                                         start=(k == 0), stop=(k == nk - 1))
                    nc.vector.tensor_copy(xw[:, m, 512 * ch:512 * (ch + 1)],
                                          pp[:, :])

        def scan_layer(whh_sb, xw, hdst):
            """Interleaved fwd/bwd 256-step scan. whh_sb/xw/hdst: dict d->tile"""
            hbuf, cbuf = {}, {}
            for d in ("f", "b"):
                hbuf[d] = rpool.tile([128, 2, KH, BL], BF16, tag=f"hbuf{d}", name=f"hbuf{d}")
                nc.vector.memset(hbuf[d][:, 0, :, :], 0.0)
                cbuf[d] = rpool.tile([128, KH, BL], F32, tag=f"cbuf{d}", name=f"cbuf{d}")
                nc.vector.memset(cbuf[d][:, :, :], 0.0)

            import contextlib
            loop_cm = (contextlib.nullcontext(0) if UNROLL == S else
                       tc.For_i(0, S, UNROLL,
                                hint_engines=(mybir.EngineType.PE,)))
            with loop_cm as i:
                for j in range(UNROLL):
                    cur, nxt = j % 2, (j + 1) % 2
                    cols = {"f": i * BL + j * BL,
                            "b": i * (-BL) + (S - 1 - j) * BL}
                    gorder = (3, 0, 1, 2)          # g, i, f, o
                    pg = {"f": {}, "b": {}}
                    gs = {"f": {}, "b": {}}
                    # interleaved f/b gate-group MMs: one 128-MM PE stream,
                    # both directions' chains overlap inside the block and
                    # their end-of-chain stalls merge into one per pair
                    for gg in gorder:
                        for d in ("f", "b"):
                            pg[d][gg] = psg.tile([128, KH, BL], F32,
                                                 tag="gates",
                                                 name=f"pg{d}{gg}")
                            for mm in range(KH):
                                m = 4 * gg + mm
                                for k in range(KH):
                                    nc.tensor.matmul(
                                        pg[d][gg][:, mm, :],
                                        whh_sb[d][:, k, 128 * m:128 * (m + 1)],
                                        hbuf[d][:, cur, k, :],
                                        start=(k == 0), stop=(k == KH - 1))
                        for d in ("f", "b"):
                            gs[d][gg] = scpool.tile([128, KH, BL], F32,
                                                    tag=f"gs{d}{gg}",
                                                    name=f"gs{d}{gg}")
                            nc.vector.tensor_tensor(
                                gs[d][gg][:, :, :], pg[d][gg][:, :, :],
                                xw[d][:, 4 * gg:4 * gg + 4, ds(cols[d], BL)],
                                OP.add)
                        if gg == 3:
                            tg = {}
                            for d in ("f", "b"):
                                tg[d] = scpool.tile([128, KH, BL], F32,
                                                    tag=f"tg{d}", name=f"tg{d}")
                                nc.scalar.activation(tg[d][:, :, :],
                                                     gs[d][3][:, :, :],
                                                     AF.Tanh)
                        elif gg == 0:
                            t1 = {}
                            for d in ("f", "b"):
                                si = scpool.tile([128, KH, BL], F32,
                                                 tag=f"si{d}")
                                nc.scalar.activation(si[:, :, :],
                                                     gs[d][0][:, :, :],
                                                     AF.Sigmoid)
                                t1[d] = scpool.tile([128, KH, BL], F32,
                                                    tag=f"t1{d}", name=f"t1{d}")
                                nc.vector.tensor_tensor(t1[d][:, :, :],
                                                        si[:, :, :],
                                                        tg[d][:, :, :],
                                                        OP.mult)
                        elif gg == 1:
                            tcc = {}
                            for d in ("f", "b"):
                                sf = scpool.tile([128, KH, BL], F32,
                                                 tag=f"sf{d}")
                                nc.scalar.activation(sf[:, :, :],
                                                     gs[d][1][:, :, :],
                                                     AF.Sigmoid)
                                t2 = scpool.tile([128, KH, BL], F32,
                                                 tag=f"t2{d}")
                                nc.vector.tensor_tensor(t2[:, :, :],
                                                        sf[:, :, :],
                                                        cbuf[d][:, :, :],
                                                        OP.mult)
                                nc.vector.tensor_tensor(cbuf[d][:, :, :],
                                                        t1[d][:, :, :],
                                                        t2[:, :, :], OP.add)
                                tcc[d] = scpool.tile([128, KH, BL], F32,
                                                     tag=f"tcc{d}",
                                                     name=f"tcc{d}")
                                nc.scalar.activation(tcc[d][:, :, :],
                                                     cbuf[d][:, :, :],
                                                     AF.Tanh)
                        else:
                            for d in ("f", "b"):
                                so = scpool.tile([128, KH, BL], F32,
                                                 tag=f"so{d}")
                                nc.scalar.activation(so[:, :, :],
                                                     gs[d][2][:, :, :],
                                                     AF.Sigmoid)
                                nc.vector.tensor_tensor(
                                    hbuf[d][:, nxt, :, :], so[:, :, :],
                                    tcc[d][:, :, :], OP.mult)
                    for d in ("f", "b"):
                        nc.vector.tensor_copy(hdst[d][:, :, ds(cols[d], BL)],
                                              hbuf[d][:, nxt, :, :])


        # ---- layer 0 -----------------------------------------------------
        whh0 = {}
        for d in ("f", "b"):
            whh0[d] = wpool.tile([128, KH, G], BF16, tag="whh", name=f"whh0{d}")
            nc.sync.dma_start(out=whh0[d][:, :, :], in_=w[f"whh0{d}"][:, :, :])
        xw0 = {}
        for d in ("f", "b"):
            xw0[d] = xwpool.tile([128, MT, SB], BF16, tag="xw", name=f"xw0{d}")
            projection(w[f"wih0{d}"], K0,
                       lambda k, ch: xt[:, k, 512 * ch:512 * (ch + 1)],
                       xw0[d])
        h0 = {d: rpool.tile([128, KH, SB], BF16, tag=f"h0{d}", name=f"h0{d}")
              for d in ("f", "b")}
        scan_layer(whh0, xw0, h0)

        # ---- layer 1 -----------------------------------------------------
        whh1 = {}
        for d in ("f", "b"):
            whh1[d] = wpool.tile([128, KH, G], BF16, tag="whh", name=f"whh1{d}")
            nc.sync.dma_start(out=whh1[d][:, :, :], in_=w[f"whh1{d}"][:, :, :])

        def l1_rhs(k, ch):
            if k < KH:
                return h0["f"][:, k, 512 * ch:512 * (ch + 1)]
            if k < 2 * KH:
                return h0["b"][:, k - KH, 512 * ch:512 * (ch + 1)]
            return ones_row[:, 512 * ch:512 * (ch + 1)]

        xw1 = {}
        for d in ("f", "b"):
            xw1[d] = xwpool.tile([128, MT, SB], BF16, tag="xw", name=f"xw1{d}")
            projection(w[f"wih1{d}"], K1, l1_rhs, xw1[d])
        h1 = {d: rpool.tile([128, KH, SB], BF16, tag=f"h1{d}", name=f"h1{d}")
              for d in ("f", "b")}
        scan_layer(whh1, xw1, h1)

        # ---- output projection ------------------------------------------
        wo = wpool.tile([128, K1, C], BF16, tag="wout")
        nc.sync.dma_start(out=wo[:, :, :], in_=w["wout"][:, :, :])
        outT = rpool.tile([C, SB], F32, tag="outT")
        for ch in range(2):
            po = pspool.tile([C, 512], F32, tag="proj")
            for k in range(K1):
                if k < KH:
                    rhs = h1["f"][:, k, 512 * ch:512 * (ch + 1)]
                elif k < 2 * KH:
                    rhs = h1["b"][:, k - KH, 512 * ch:512 * (ch + 1)]
                else:
                    rhs = ones_row[:, 512 * ch:512 * (ch + 1)]
                nc.tensor.matmul(po[:, :], wo[:, k, :], rhs,
                                 start=(k == 0), stop=(k == K1 - 1))
            nc.vector.tensor_copy(outT[:, 512 * ch:512 * (ch + 1)], po[:, :])
        out_sbc = out.rearrange("b s c -> s b c")
        for cb in range(8):
            pt = pspool.tile([128, C], F32, tag="tp")
            nc.tensor.transpose(pt[:, :], outT[:, 128 * cb:128 * (cb + 1)],
                                ident[0:C, 0:C])
            onat = scpool.tile([128, C], F32, tag="onat")
            nc.vector.tensor_copy(onat[:, :], pt[:, :])
            nc.sync.dma_start(out=out_sbc[32 * cb:32 * (cb + 1), :, :],
                              in_=onat[:, :])


def _get_nc():
    if "nc" not in _cache:
        _cache["nc"] = build_nc()
    return _cache["nc"]


def kernel(**inputs):
    from concourse.bass_utils import run_bass_kernel_spmd

    wmaps = _prep_weights(inputs)
    hsf = np.asarray(inputs["hidden_states"], np.float32)
    rol = np.asarray(inputs["roles"])
    prd = np.asarray(inputs["predicates"])
    in_maps = []
    for c in range(NCORES):
        sl = slice(BL * c, BL * (c + 1))
        m = dict(wmaps)
        m["hs"] = _bf(hsf[:, sl])                                   # [4,BL,S,E]
        m["roles"] = np.ascontiguousarray(
            rol[sl].T.reshape(1, SB)).astype(np.float32)            # (t,b)
        m["preds"] = np.ascontiguousarray(
            prd[sl].T.reshape(1, SB)).astype(np.float32)
        in_maps.append(m)

    nc = _get_nc()
    res = run_bass_kernel_spmd(nc, in_maps, core_ids=list(range(NCORES)))
    return np.concatenate([r["out"] for r in res.results], axis=0)



# revision 23
# speedup vs baseline: 1.4648x; 1.4648x over previous
"""Trainium2 Bass kernel for nn_ArgumentClassification (2-layer BiLSTM tagger).

Sharding: 8 cores = 4 batch slices x 2 directions. Core c handles batch rows
[c//2*8 : c//2*8+8] and direction ('f' if c%2==0 else 'b'). Backward cores
receive their inputs TIME-REVERSED on the host, so the device program is
identical on every core (pure forward scan); the host un-reverses and sums
the per-direction partial outputs.

This halves the per-core scan weight-load volume vs. batch-only sharding:
the 256-step LSTM recurrence is bound by streaming Whh (2048x512 bf16)
through the PE array every step (~45ns per ldweights+matmul pair), so one
direction per core = 64 pairs/step instead of 128.

Mid-kernel exchange: layer 1 consumes [h0f; h0b]. Each core stores a
time-reversed copy of its layer-0 output (hdst2) -- reversed-in-my-frame is
exactly the peer's time convention -- and the fwd/bwd core pairs AllReduce
their hdst2 through HBM; subtracting one's own contribution leaves the
peer's h0, time-aligned locally. The output projection splits by k:
out = h1f @ Wf.T + (h1b @ Wb.T reversed), summed on the host, so no second
exchange is needed.

Per-core pipeline:
  1. x.T features [128, 7, SB] built from the mean of 4 transformer layers
     (PE transposes), plus delta/mask/ones feature rows. The predicate
     one-hot and role mask are host-precomputed (tiny [B,S] int ops).
  2. L0 input projection (batched matmuls, bias folded via ones-row).
  3. L0 scan: 256 steps in gates-transposed layout [2048, BL], Whh
     stationary, gate groups in (g,i,f,o) order so the DVE/ACT nonlinearity
     chain of each group overlaps the next group's matmuls.
  4. hdst2 AllReduce with pair core; hrecv = sum - hdst2.
  5. L1 projection from [hdst(own); hrecv(peer)] + bias, L1 scan.
  6. out partial = h1 @ W_out[own half].T (+ bias on fwd cores only),
     PE-transposed to [BL, S, 30] and DMA'd out.

Gate order is host-permuted from PyTorch's (i,f,g,o) to (i,f,o,g).
"""
import sys

sys.path.insert(0, "/opt/trn_rl_repo")

import numpy as np
import ml_dtypes

import concourse.bass as bass
import concourse.tile as tile
from concourse import bacc, mybir
from concourse.bass import ds
from concourse.masks import make_identity

BF16 = mybir.dt.bfloat16
F32 = mybir.dt.float32
AF = mybir.ActivationFunctionType
OP = mybir.AluOpType

B, S, E, H, C = 32, 256, 768, 512, 30
NCORES = 8
NPAIR = 4                 # batch slices (pairs of cores)
BL = B // NPAIR           # 8 rows per core
SB = S * BL               # 2048 columns, ordered (t, b): col = t*BL + b
G = 4 * H                 # 2048 gate rows
MT = G // 128             # 16 gate m-tiles
KH = H // 128             # 4 hidden k-tiles
K0 = 7                    # L0 input k-tiles ([770 + ones-row] padded to 896)
K1O = 5                   # L1 own-half k-tiles (512 + bias row -> 640)
K1R = 4                   # L1 recv-half k-tiles (512)
RG = [[0, 1], [2, 3], [4, 5], [6, 7]]

_cache = {}


def _bf(a):
    return np.asarray(a, dtype=ml_dtypes.bfloat16)


def _prep_weights(inp, d):
    """Host-side weight prep for direction d ('f'/'b'): permute gates to
    (i,f,o,g), transpose, pad, fold biases, tile for SBUF."""
    perm = np.concatenate([
        np.arange(0, H),          # i
        np.arange(H, 2 * H),      # f
        np.arange(3 * H, 4 * H),  # o
        np.arange(2 * H, 3 * H),  # g
    ])
    out = {}

    def tile_k(a, nk):
        # [nk*128, M] -> [128, nk, M]
        return np.ascontiguousarray(
            a.reshape(nk, 128, a.shape[1]).transpose(1, 0, 2))

    def tile_km(a, nk):
        # [nk*128, 16*128] -> [16, 128, nk, 128]  (per-m-block contiguous)
        m = a.shape[1] // 128
        return np.ascontiguousarray(
            a.reshape(nk, 128, m, 128).transpose(2, 1, 0, 3))

    own = slice(0, H) if d == "f" else slice(H, 2 * H)
    rcv = slice(H, 2 * H) if d == "f" else slice(0, H)

    # layer 0
    wih = inp[f"Wih_l0{d}"][perm]                     # [2048, 770]
    bias = (inp[f"bih_l0{d}"] + inp[f"bhh_l0{d}"])[perm]
    ext = np.zeros((K0 * 128, G), np.float32)
    ext[:768] = wih.T[:768]
    ext[768] = wih.T[768]      # delta coeffs at tile6 partition 0
    ext[800] = wih.T[769]      # mask coeffs at tile6 partition 32
    ext[832] = bias            # bias row at tile6 partition 64
    out["wih0"] = _bf(tile_km(ext, K0))               # [16,128,7,128]
    whh = inp[f"Whh_l0{d}"][perm]                     # [2048, 512]
    out["whh0"] = _bf(tile_k(whh.T, KH))              # [128, 4, 2048]

    # layer 1, split into own-half (+bias) and recv-half
    w1T = inp[f"Wih_l1{d}"][perm].T                   # [1024, 2048]
    bias = (inp[f"bih_l1{d}"] + inp[f"bhh_l1{d}"])[perm]
    ext = np.zeros((K1O * 128, G), np.float32)
    ext[:512] = w1T[own]
    ext[512] = bias
    out["wih1o"] = _bf(tile_km(ext, K1O))             # [16,128,5,128]
    ext = np.zeros((K1R * 128, G), np.float32)
    ext[:512] = w1T[rcv]
    out["wih1r"] = _bf(tile_km(ext, K1R))             # [16,128,4,128]
    whh = inp[f"Whh_l1{d}"][perm]
    out["whh1"] = _bf(tile_k(whh.T, KH))

    # output projection own half [640, 30]; bias only on fwd cores
    ext = np.zeros((K1O * 128, C), np.float32)
    ext[:512] = inp["W_out"].T[own]
    if d == "f":
        ext[512] = inp["b_out"]
    out["wout"] = _bf(tile_k(ext, K1O))               # [128, 5, 30]
    return out


def _prep_core_inputs(inputs, wmaps, core):
    pair, parity = core // 2, core % 2
    d = "f" if parity == 0 else "b"
    rows = slice(BL * pair, BL * (pair + 1))

    hs = np.asarray(inputs["hidden_states"], np.float32)[:, rows]  # [4,BL,S,E]
    roles = np.asarray(inputs["roles"])[rows]                      # [BL,S]
    preds = np.asarray(inputs["predicates"])[rows]
    rmask = ((roles != 0) & (roles != -100)).astype(np.float32)
    idx = np.argmax(preds, axis=-1)                                # [BL]
    mw = hs.mean(axis=0).mean(axis=-1)                             # [BL,S]
    delta = (mw - np.take_along_axis(mw, idx[:, None], 1)).astype(np.float32)
    if parity == 1:  # time-reverse for backward cores
        hs = hs[:, :, ::-1]
        rmask = rmask[:, ::-1]
        delta = delta[:, ::-1]
    m = dict(wmaps[d])
    m["hs"] = _bf(hs)
    m["rmask"] = np.ascontiguousarray(rmask.T).reshape(1, SB)      # (t,b)
    m["drow"] = np.ascontiguousarray(delta.T).reshape(1, SB)
    return m


def build_nc():
    nc = bacc.Bacc("TRN2", target_bir_lowering=False, debug=False,
                   num_devices=NCORES)
    hs = nc.dram_tensor("hs", [4, BL, S, E], BF16, kind="ExternalInput").ap()
    rmask = nc.dram_tensor("rmask", [1, SB], F32, kind="ExternalInput").ap()
    drow = nc.dram_tensor("drow", [1, SB], F32, kind="ExternalInput").ap()
    w = {}
    w["wih0"] = nc.dram_tensor("wih0", [MT, 128, K0, 128], BF16,
                               kind="ExternalInput").ap()
    w["wih1o"] = nc.dram_tensor("wih1o", [MT, 128, K1O, 128], BF16,
                                kind="ExternalInput").ap()
    w["wih1r"] = nc.dram_tensor("wih1r", [MT, 128, K1R, 128], BF16,
                                kind="ExternalInput").ap()
    w["whh0"] = nc.dram_tensor("whh0", [128, KH, G], BF16,
                               kind="ExternalInput").ap()
    w["whh1"] = nc.dram_tensor("whh1", [128, KH, G], BF16,
                               kind="ExternalInput").ap()
    w["wout"] = nc.dram_tensor("wout", [128, K1O, C], BF16,
                               kind="ExternalInput").ap()
    hx_in = nc.dram_tensor("hx_in", [128, KH, SB], BF16, kind="Internal")
    hx_out = nc.dram_tensor("hx_out", [128, KH, SB], BF16, kind="Internal")
    out = nc.dram_tensor("out", [BL, S, C], F32, kind="ExternalOutput").ap()

    with tile.TileContext(nc) as tc:
        _emit(nc, tc, hs, rmask, drow, w, hx_in, hx_out, out)
    nc.compile()
    return nc


def _emit(nc, tc, hs, rmask, drow, w, hx_in, hx_out, out):
    from contextlib import ExitStack
    with ExitStack() as st:
        cpool = st.enter_context(tc.tile_pool(name="const", bufs=1))
        rpool = st.enter_context(tc.tile_pool(name="rows", bufs=1))
        xwpool = st.enter_context(tc.tile_pool(name="xw", bufs=1))
        scpool = st.enter_context(tc.tile_pool(name="sc", bufs=3))
        wpool = st.enter_context(tc.tile_pool(name="wts", bufs=2))
        pspool = st.enter_context(tc.tile_pool(name="ps", bufs=1, space="PSUM"))
        psproj = st.enter_context(tc.tile_pool(name="psp", bufs=2,
                                               space="PSUM"))
        psg = st.enter_context(tc.tile_pool(name="psg", bufs=5, space="PSUM"))

        ident = cpool.tile([128, 128], F32, tag="ident")
        make_identity(nc, ident[:, :])
        ones_row = cpool.tile([128, SB], BF16, tag="onesrow")
        nc.vector.memset(ones_row[:, :], 0.0)
        nc.vector.memset(ones_row[0:1, :], 1.0)

        whh0 = wpool.tile([128, KH, G], BF16, tag="whh", name="whh0")
        nc.sync.dma_start(out=whh0[:, :, :], in_=w["whh0"][:, :, :])
        hdst = rpool.tile([128, KH, SB], BF16, tag="hdst", name="hdst")
        hdst2 = rpool.tile([128, KH, SB], BF16, tag="hdst2", name="hdst2")

        # ---- projections + scan -----------------------------------------
        def projection(segs, xw):
            """xw[:, m, :] (bf16 [128, MT, SB]) = sum over segments of
            Wseg.T @ rhs_seg. segs: list of (wih_dram, nk, rhs_of_k)."""
            for m in range(MT):
                wms = []
                for (wih_dram, nk, _) in segs:
                    wm = wpool.tile([128, nk, 128], BF16, tag=f"wihm{nk}")
                    nc.sync.dma_start(out=wm[:, :, :], in_=wih_dram[m])
                    wms.append(wm)
                for ch in range(SB // 512):
                    pp = psproj.tile([128, 512], F32, tag="proj")
                    nks = sum(s[1] for s in segs)
                    kk = 0
                    for wm, (_, nk, rhs_of_k) in zip(wms, segs):
                        for k in range(nk):
                            nc.tensor.matmul(pp[:, :], wm[:, k, :],
                                             rhs_of_k(k, ch),
                                             start=(kk == 0),
                                             stop=(kk == nks - 1))
                            kk += 1
                    nc.vector.tensor_copy(xw[:, m, 512 * ch:512 * (ch + 1)],
                                          pp[:, :])

        def scan_layer(whh_sb, xw, hdst, hdst2):
            """Single-direction 256-step scan."""
            hbuf = rpool.tile([128, 2, KH, BL], BF16, tag="hbuf", name="hbuf")
            nc.vector.memset(hbuf[:, 0, :, :], 0.0)
            cbuf = rpool.tile([128, KH, BL], F32, tag="cbuf", name="cbuf")
            nc.vector.memset(cbuf[:, :, :], 0.0)

            for j in range(S):
                cur, nxt = j % 2, (j + 1) % 2
                cols = j * BL
                gorder = (3, 0, 1, 2)          # g, i, f, o
                pg, gs = {}, {}
                for gg in gorder:
                    pg[gg] = psg.tile([128, KH, BL], F32, tag="gates",
                                      name=f"pg{gg}")
                    for mm in range(KH):
                        m = 4 * gg + mm
                        for k in range(KH):
                            nc.tensor.matmul(
                                pg[gg][:, mm, :],
                                whh_sb[:, k, 128 * m:128 * (m + 1)],
                                hbuf[:, cur, k, :],
                                start=(k == 0), stop=(k == KH - 1))
                    gs[gg] = scpool.tile([128, KH, BL], F32, tag=f"gs{gg}",
                                         name=f"gs{gg}")
                    nc.vector.tensor_tensor(
                        gs[gg][:, :, :], pg[gg][:, :, :],
                        xw[:, 4 * gg:4 * gg + 4, ds(cols, BL)], OP.add)
                    if gg == 3:
                        tg = scpool.tile([128, KH, BL], F32, tag="tg",
                                         name="tg")
                        nc.scalar.activation(tg[:, :, :], gs[3][:, :, :],
                                             AF.Tanh)
                    elif gg == 0:
                        si = scpool.tile([128, KH, BL], F32, tag="si")
                        nc.scalar.activation(si[:, :, :], gs[0][:, :, :],
                                             AF.Sigmoid)
                        t1 = scpool.tile([128, KH, BL], F32, tag="t1",
                                         name="t1")
                        nc.vector.tensor_tensor(t1[:, :, :], si[:, :, :],
                                                tg[:, :, :], OP.mult)
                    elif gg == 1:
                        sf = scpool.tile([128, KH, BL], F32, tag="sf")
                        nc.scalar.activation(sf[:, :, :], gs[1][:, :, :],
                                             AF.Sigmoid)
                        t2 = scpool.tile([128, KH, BL], F32, tag="t2")
                        nc.vector.tensor_tensor(t2[:, :, :], sf[:, :, :],
                                                cbuf[:, :, :], OP.mult)
                        nc.vector.tensor_tensor(cbuf[:, :, :], t1[:, :, :],
                                                t2[:, :, :], OP.add)
                        tcc = scpool.tile([128, KH, BL], F32, tag="tcc",
                                          name="tcc")
                        nc.scalar.activation(tcc[:, :, :], cbuf[:, :, :],
                                             AF.Tanh)
                    else:
                        so = scpool.tile([128, KH, BL], F32, tag="so",
                                         name="so")
                        nc.scalar.activation(so[:, :, :], gs[2][:, :, :],
                                             AF.Sigmoid)
                        nc.vector.tensor_tensor(hbuf[:, nxt, :, :],
                                                so[:, :, :], tcc[:, :, :],
                                                OP.mult)
                nc.vector.tensor_tensor(hdst[:, :, ds(cols, BL)],
                                        so[:, :, :], tcc[:, :, :], OP.mult)
                if hdst2 is not None:
                    nc.vector.tensor_tensor(
                        hdst2[:, :, ds((S - 1 - j) * BL, BL)],
                        so[:, :, :], tcc[:, :, :], OP.mult)

        # ---- layer 0: x.T features + projection (short-lived pools) ------
        xw0 = xwpool.tile([128, MT, SB], BF16, tag="xw", name="xw0")
        NRT = SB // 128  # 16 row-tiles of (t,b)
        TPR = 128 // BL  # 16 timesteps per row-tile
        with tc.tile_pool(name="xtp", bufs=1) as xtpool, \
                tc.tile_pool(name="hl", bufs=5) as hlpool, \
                tc.tile_pool(name="sum", bufs=3) as sumpool, \
                tc.tile_pool(name="frp", bufs=1) as frpool:
            xt = xtpool.tile([128, K0, SB], BF16, tag="xt")
            hs_sbe = hs.rearrange("l b s e -> l s b e")
            for r in range(NRT):
                hl = []
                for layer in range(4):
                    t = hlpool.tile([128, E], BF16, tag="hl")
                    nc.sync.dma_start(
                        out=t[:, :],
                        in_=hs_sbe[layer, TPR * r:TPR * (r + 1), :, :])
                    hl.append(t)
                s01 = sumpool.tile([128, E], F32, tag="sum")
                nc.vector.tensor_tensor(s01[:, :], hl[0][:, :], hl[1][:, :],
                                        OP.add)
                s23 = sumpool.tile([128, E], F32, tag="sum")
                nc.vector.tensor_tensor(s23[:, :], hl[2][:, :], hl[3][:, :],
                                        OP.add)
                ssum = sumpool.tile([128, E], F32, tag="sum")
                nc.vector.tensor_tensor(ssum[:, :], s01[:, :], s23[:, :],
                                        OP.add)
                for c in range(6):
                    pt = pspool.tile([128, 128], F32, tag="tp")
                    nc.tensor.transpose(pt[:, :],
                                        ssum[:, 128 * c:128 * (c + 1)],
                                        ident[:, :])
                    nc.vector.tensor_scalar_mul(
                        xt[:, c, 128 * r:128 * (r + 1)], pt[:, :], 0.25)

            # feature rows (delta@p0, mask@p32, ones@p64) in xt[:, 6, :]
            nc.vector.memset(xt[:, 6, :], 0.0)
            nc.vector.memset(xt[64:65, 6, :], 1.0)
            frow = frpool.tile([1, SB], F32, tag="frow", name="frow_r")
            nc.sync.dma_start(out=frow[:, :], in_=rmask[:, :])
            nc.vector.tensor_copy(xt[32:33, 6, :], frow[:, :])
            frow2 = frpool.tile([1, SB], F32, tag="frow", name="frow_d")
            nc.sync.dma_start(out=frow2[:, :], in_=drow[:, :])
            nc.vector.tensor_copy(xt[0:1, 6, :], frow2[:, :])

            projection([(w["wih0"], K0,
                         lambda k, ch: xt[:, k, 512 * ch:512 * (ch + 1)])],
                       xw0)

        scan_layer(whh0, xw0, hdst, hdst2)

        latepool = st.enter_context(tc.tile_pool(name="late", bufs=1))

        # ---- exchange: AllReduce pair; hrecv = sum - own ----------------
        nc.sync.dma_start(out=hx_in.ap(), in_=hdst2[:, :, :])
        nc.gpsimd.collective_compute(
            "AllReduce", OP.add, replica_groups=RG,
            ins=[hx_in.ap().opt()], outs=[hx_out.ap().opt()])
        hrecv = latepool.tile([128, KH, SB], BF16, tag="hrecv", name="hrecv")
        nc.sync.dma_start(out=hrecv[:, :, :], in_=hx_out.ap())
        nc.vector.tensor_tensor(hrecv[:, :, :], hrecv[:, :, :],
                                hdst2[:, :, :], OP.subtract)

        # ---- layer 1 -----------------------------------------------------
        whh1 = wpool.tile([128, KH, G], BF16, tag="whh", name="whh1")
        nc.sync.dma_start(out=whh1[:, :, :], in_=w["whh1"][:, :, :])

        def own_rhs(k, ch):
            if k < KH:
                return hdst[:, k, 512 * ch:512 * (ch + 1)]
            return ones_row[:, 512 * ch:512 * (ch + 1)]

        def rcv_rhs(k, ch):
            return hrecv[:, k, 512 * ch:512 * (ch + 1)]

        xw1 = xwpool.tile([128, MT, SB], BF16, tag="xw", name="xw1")
        projection([(w["wih1o"], K1O, own_rhs),
                    (w["wih1r"], K1R, rcv_rhs)], xw1)
        h1 = latepool.tile([128, KH, SB], BF16, tag="h1", name="h1")
        scan_layer(whh1, xw1, h1, None)

        # ---- output projection ------------------------------------------
        wo = wpool.tile([128, K1O, C], BF16, tag="wout")
        nc.sync.dma_start(out=wo[:, :, :], in_=w["wout"][:, :, :])
        out_sbc = out.rearrange("b s c -> s b c")
        for ch in range(SB // 512):
            po = psproj.tile([C, 512], F32, tag="proj")
            for k in range(K1O):
                if k < KH:
                    rhs = h1[:, k, 512 * ch:512 * (ch + 1)]
                else:
                    rhs = ones_row[:, 512 * ch:512 * (ch + 1)]
                nc.tensor.matmul(po[:, :], wo[:, k, :], rhs,
                                 start=(k == 0), stop=(k == K1O - 1))
            ost = scpool.tile([C, 512], F32, tag="ost")
            nc.vector.tensor_copy(ost[:, :], po[:, :])
            for cb in range(4):
                pt = pspool.tile([128, C], F32, tag="tp")
                nc.tensor.transpose(pt[:, :], ost[:, 128 * cb:128 * (cb + 1)],
                                    ident[0:C, 0:C])
                onat = scpool.tile([128, C], F32, tag="onat")
                nc.vector.tensor_copy(onat[:, :], pt[:, :])
                gb = 4 * ch + cb
                nc.sync.dma_start(out=out_sbc[TPR * gb:TPR * (gb + 1), :, :],
                                  in_=onat[:, :])


def _get_nc():
    if "nc" not in _cache:
        _cache["nc"] = build_nc()
    return _cache["nc"]


def make_in_maps(inputs):
    wmaps = {d: _prep_weights(inputs, d) for d in ("f", "b")}
    return [_prep_core_inputs(inputs, wmaps, c) for c in range(NCORES)]


def kernel(**inputs):
    from concourse.bass_utils import run_bass_kernel_spmd

    in_maps = make_in_maps(inputs)
    nc = _get_nc()
    res = run_bass_kernel_spmd(nc, in_maps, core_ids=list(range(NCORES)))
    parts = [r["out"] for r in res.results]
    full = np.empty((B, S, C), np.float32)
    for p in range(NPAIR):
        full[BL * p:BL * (p + 1)] = parts[2 * p] + parts[2 * p + 1][:, ::-1]
    return full


# revision 26
# speedup vs baseline: 1.5537x; 1.0606x over previous
"""Trainium2 Bass kernel for nn_ArgumentClassification (2-layer BiLSTM tagger).

Sharding: 8 cores = 4 batch slices x 2 directions. Core c handles batch rows
[c//2*8 : c//2*8+8] and direction ('f' if c%2==0 else 'b'). Backward cores
receive their inputs TIME-REVERSED on the host, so the device program is
identical on every core (pure forward scan); the host un-reverses and sums
the per-direction partial outputs.

This halves the per-core scan weight-load volume vs. batch-only sharding:
the 256-step LSTM recurrence is bound by streaming Whh (2048x512 bf16)
through the PE array every step (~45ns per ldweights+matmul pair), so one
direction per core = 64 pairs/step instead of 128.

Mid-kernel exchange: layer 1 consumes [h0f; h0b]. Each core stores a
time-reversed copy of its layer-0 output (hdst2) -- reversed-in-my-frame is
exactly the peer's time convention -- and the fwd/bwd core pairs AllReduce
their hdst2 through HBM; subtracting one's own contribution leaves the
peer's h0, time-aligned locally. The output projection splits by k:
out = h1f @ Wf.T + (h1b @ Wb.T reversed), summed on the host, so no second
exchange is needed.

Per-core pipeline:
  1. x.T features [128, 7, SB] built from the mean of 4 transformer layers
     (PE transposes), plus delta/mask/ones feature rows. The predicate
     one-hot and role mask are host-precomputed (tiny [B,S] int ops).
  2. L0 input projection (batched matmuls, bias folded via ones-row).
  3. L0 scan: 256 steps in gates-transposed layout [2048, BL], Whh
     stationary, gate groups in (g,i,f,o) order so the DVE/ACT nonlinearity
     chain of each group overlaps the next group's matmuls.
  4. hdst2 AllReduce with pair core; hrecv = sum - hdst2.
  5. L1 projection from [hdst(own); hrecv(peer)] + bias, L1 scan.
  6. out partial = h1 @ W_out[own half].T (+ bias on fwd cores only),
     PE-transposed to [BL, S, 30] and DMA'd out.

Gate order is host-permuted from PyTorch's (i,f,g,o) to (i,f,o,g).
"""
import sys

sys.path.insert(0, "/opt/trn_rl_repo")

import numpy as np
import ml_dtypes

import concourse.bass as bass
import concourse.tile as tile
from concourse import bacc, mybir
from concourse.bass import ds
from concourse.masks import make_identity

BF16 = mybir.dt.bfloat16
F32 = mybir.dt.float32
AF = mybir.ActivationFunctionType
OP = mybir.AluOpType

B, S, E, H, C = 32, 256, 768, 512, 30
NCORES = 8
NPAIR = 4                 # batch slices (pairs of cores)
BL = B // NPAIR           # 8 rows per core
SB = S * BL               # 2048 columns, ordered (t, b): col = t*BL + b
G = 4 * H                 # 2048 gate rows
MT = G // 128             # 16 gate m-tiles
KH = H // 128             # 4 hidden k-tiles
K0 = 7                    # L0 input k-tiles ([770 + ones-row] padded to 896)
K1O = 5                   # L1 own-half k-tiles (512 + bias row -> 640)
K1R = 4                   # L1 recv-half k-tiles (512)
RG = [[0, 1], [2, 3], [4, 5], [6, 7]]

_cache = {}


def _bf(a):
    return np.asarray(a, dtype=ml_dtypes.bfloat16)


def _prep_weights(inp, d):
    """Host-side weight prep for direction d ('f'/'b'): permute gates to
    (i,f,o,g), transpose, pad, fold biases, tile for SBUF."""
    perm = np.concatenate([
        np.arange(0, H),          # i
        np.arange(H, 2 * H),      # f
        np.arange(3 * H, 4 * H),  # o
        np.arange(2 * H, 3 * H),  # g
    ])
    out = {}

    def tile_k(a, nk):
        # [nk*128, M] -> [128, nk, M]
        return np.ascontiguousarray(
            a.reshape(nk, 128, a.shape[1]).transpose(1, 0, 2))

    def tile_km(a, nk):
        # [nk*128, 16*128] -> [16, 128, nk, 128]  (per-m-block contiguous)
        m = a.shape[1] // 128
        return np.ascontiguousarray(
            a.reshape(nk, 128, m, 128).transpose(2, 1, 0, 3))

    own = slice(0, H) if d == "f" else slice(H, 2 * H)
    rcv = slice(H, 2 * H) if d == "f" else slice(0, H)

    # layer 0
    wih = inp[f"Wih_l0{d}"][perm]                     # [2048, 770]
    bias = (inp[f"bih_l0{d}"] + inp[f"bhh_l0{d}"])[perm]
    ext = np.zeros((K0 * 128, G), np.float32)
    ext[:768] = wih.T[:768]
    ext[768] = wih.T[768]      # delta coeffs at tile6 partition 0
    ext[800] = wih.T[769]      # mask coeffs at tile6 partition 32
    ext[832] = bias            # bias row at tile6 partition 64
    out["wih0"] = _bf(tile_km(ext, K0))               # [16,128,7,128]
    whh = inp[f"Whh_l0{d}"][perm]                     # [2048, 512]
    out["whh0"] = _bf(tile_k(whh.T, KH))              # [128, 4, 2048]

    # layer 1, split into own-half (+bias) and recv-half
    w1T = inp[f"Wih_l1{d}"][perm].T                   # [1024, 2048]
    bias = (inp[f"bih_l1{d}"] + inp[f"bhh_l1{d}"])[perm]
    ext = np.zeros((K1O * 128, G), np.float32)
    ext[:512] = w1T[own]
    ext[512] = bias
    out["wih1o"] = _bf(tile_km(ext, K1O))             # [16,128,5,128]
    ext = np.zeros((K1R * 128, G), np.float32)
    ext[:512] = w1T[rcv]
    out["wih1r"] = _bf(tile_km(ext, K1R))             # [16,128,4,128]
    whh = inp[f"Whh_l1{d}"][perm]
    out["whh1"] = _bf(tile_k(whh.T, KH))

    # output projection own half [640, 30]; bias only on fwd cores
    ext = np.zeros((K1O * 128, C), np.float32)
    ext[:512] = inp["W_out"].T[own]
    if d == "f":
        ext[512] = inp["b_out"]
    out["wout"] = _bf(tile_k(ext, K1O))               # [128, 5, 30]
    return out


def _prep_core_inputs(inputs, wmaps, core):
    pair, parity = core // 2, core % 2
    d = "f" if parity == 0 else "b"
    rows = slice(BL * pair, BL * (pair + 1))

    hs = np.asarray(inputs["hidden_states"], np.float32)[:, rows]  # [4,BL,S,E]
    roles = np.asarray(inputs["roles"])[rows]                      # [BL,S]
    preds = np.asarray(inputs["predicates"])[rows]
    rmask = ((roles != 0) & (roles != -100)).astype(np.float32)
    idx = np.argmax(preds, axis=-1)                                # [BL]
    mw = hs.mean(axis=0).mean(axis=-1)                             # [BL,S]
    delta = (mw - np.take_along_axis(mw, idx[:, None], 1)).astype(np.float32)
    if parity == 1:  # time-reverse for backward cores
        hs = hs[:, :, ::-1]
        rmask = rmask[:, ::-1]
        delta = delta[:, ::-1]
    m = dict(wmaps[d])
    m["hs"] = _bf(hs)
    m["rmask"] = np.ascontiguousarray(rmask.T).reshape(1, SB)      # (t,b)
    m["drow"] = np.ascontiguousarray(delta.T).reshape(1, SB)
    return m


def build_nc():
    nc = bacc.Bacc("TRN2", target_bir_lowering=False, debug=False,
                   num_devices=NCORES)
    hs = nc.dram_tensor("hs", [4, BL, S, E], BF16, kind="ExternalInput").ap()
    rmask = nc.dram_tensor("rmask", [1, SB], F32, kind="ExternalInput").ap()
    drow = nc.dram_tensor("drow", [1, SB], F32, kind="ExternalInput").ap()
    w = {}
    w["wih0"] = nc.dram_tensor("wih0", [MT, 128, K0, 128], BF16,
                               kind="ExternalInput").ap()
    w["wih1o"] = nc.dram_tensor("wih1o", [MT, 128, K1O, 128], BF16,
                                kind="ExternalInput").ap()
    w["wih1r"] = nc.dram_tensor("wih1r", [MT, 128, K1R, 128], BF16,
                                kind="ExternalInput").ap()
    w["whh0"] = nc.dram_tensor("whh0", [128, KH, G], BF16,
                               kind="ExternalInput").ap()
    w["whh1"] = nc.dram_tensor("whh1", [128, KH, G], BF16,
                               kind="ExternalInput").ap()
    w["wout"] = nc.dram_tensor("wout", [128, K1O, C], BF16,
                               kind="ExternalInput").ap()
    hx_in = nc.dram_tensor("hx_in", [128, KH, SB], BF16, kind="Internal")
    hx_out = nc.dram_tensor("hx_out", [128, KH, SB], BF16, kind="Internal")
    out = nc.dram_tensor("out", [BL, S, C], F32, kind="ExternalOutput").ap()

    with tile.TileContext(nc) as tc:
        _emit(nc, tc, hs, rmask, drow, w, hx_in, hx_out, out)
    nc.compile()
    return nc


class _Filler:
    """Deadline-aware FIFO of emission thunks. Items are emitted between
    scan steps so their PE work lands in the scan's dependency-stall gaps.
    Strict FIFO pops keep PE program order consistent with producer ->
    consumer order (no in-order-engine deadlocks)."""

    def __init__(self):
        self.q = []

    def add(self, earliest, deadline, fn):
        self.q.append((earliest, deadline, fn))

    def step(self, j, budget=1):
        n = 0
        while self.q and self.q[0][1] <= j:
            self.q.pop(0)[2]()
            n += 1
        while self.q and n < budget and self.q[0][0] <= j:
            self.q.pop(0)[2]()
            n += 1

    def drain(self):
        while self.q:
            self.q.pop(0)[2]()


def _emit(nc, tc, hs, rmask, drow, w, hx_in, hx_out, out):
    from contextlib import ExitStack
    NCH = SB // 512          # 4 column chunks (64 timesteps each)
    SPC = S // NCH           # 64 scan steps per chunk
    NRT = SB // 128          # 16 (t,b) row-tiles
    RPC = NRT // NCH         # 4 row-tiles per chunk
    TPR = 128 // BL          # 16 timesteps per row-tile
    with ExitStack() as st:
        cpool = st.enter_context(tc.tile_pool(name="const", bufs=1))
        rpool = st.enter_context(tc.tile_pool(name="rows", bufs=1))
        xwpool = st.enter_context(tc.tile_pool(name="xw", bufs=1))
        scpool = st.enter_context(tc.tile_pool(name="sc", bufs=3))
        wpool = st.enter_context(tc.tile_pool(name="wts", bufs=2))
        pspool = st.enter_context(tc.tile_pool(name="ps", bufs=1, space="PSUM"))
        psproj = st.enter_context(tc.tile_pool(name="psp", bufs=2,
                                               space="PSUM"))
        psg = st.enter_context(tc.tile_pool(name="psg", bufs=5, space="PSUM"))

        ident = cpool.tile([128, 128], F32, tag="ident")
        make_identity(nc, ident[:, :])
        ones_row = cpool.tile([128, SB], BF16, tag="onesrow")
        nc.vector.memset(ones_row[:, :], 0.0)
        nc.vector.memset(ones_row[0:1, :], 1.0)

        whh0 = wpool.tile([128, KH, G], BF16, tag="whh", name="whh0")
        nc.sync.dma_start(out=whh0[:, :, :], in_=w["whh0"][:, :, :])
        hdst = rpool.tile([128, KH, SB], BF16, tag="hdst", name="hdst")
        hdst2 = rpool.tile([128, KH, SB], BF16, tag="hdst2", name="hdst2")

        filler = _Filler()

        def scan_layer(whh_sb, xw, hd, hd2):
            """Single-direction 256-step scan with gap-filler items."""
            hbuf = rpool.tile([128, 2, KH, BL], BF16, tag="hbuf", name="hbuf")
            nc.vector.memset(hbuf[:, 0, :, :], 0.0)
            cbuf = rpool.tile([128, KH, BL], F32, tag="cbuf", name="cbuf")
            nc.vector.memset(cbuf[:, :, :], 0.0)

            for j in range(S):
                filler.step(j)
                cur, nxt = j % 2, (j + 1) % 2
                cols = j * BL
                gorder = (3, 0, 1, 2)          # g, i, f, o
                pg, gs = {}, {}
                for gg in gorder:
                    pg[gg] = psg.tile([128, KH, BL], F32, tag="gates",
                                      name=f"pg{gg}")
                    for mm in range(KH):
                        m = 4 * gg + mm
                        for k in range(KH):
                            nc.tensor.matmul(
                                pg[gg][:, mm, :],
                                whh_sb[:, k, 128 * m:128 * (m + 1)],
                                hbuf[:, cur, k, :],
                                start=(k == 0), stop=(k == KH - 1))
                    gs[gg] = scpool.tile([128, KH, BL], F32, tag=f"gs{gg}",
                                         name=f"gs{gg}")
                    nc.vector.tensor_tensor(
                        gs[gg][:, :, :], pg[gg][:, :, :],
                        xw[:, 4 * gg:4 * gg + 4, ds(cols, BL)], OP.add)
                    if gg == 3:
                        tg = scpool.tile([128, KH, BL], F32, tag="tg",
                                         name="tg")
                        nc.scalar.activation(tg[:, :, :], gs[3][:, :, :],
                                             AF.Tanh)
                    elif gg == 0:
                        si = scpool.tile([128, KH, BL], F32, tag="si")
                        nc.scalar.activation(si[:, :, :], gs[0][:, :, :],
                                             AF.Sigmoid)
                        t1 = scpool.tile([128, KH, BL], F32, tag="t1",
                                         name="t1")
                        nc.vector.tensor_tensor(t1[:, :, :], si[:, :, :],
                                                tg[:, :, :], OP.mult)
                    elif gg == 1:
                        sf = scpool.tile([128, KH, BL], F32, tag="sf")
                        nc.scalar.activation(sf[:, :, :], gs[1][:, :, :],
                                             AF.Sigmoid)
                        t2 = scpool.tile([128, KH, BL], F32, tag="t2")
                        nc.vector.tensor_tensor(t2[:, :, :], sf[:, :, :],
                                                cbuf[:, :, :], OP.mult)
                        nc.vector.tensor_tensor(cbuf[:, :, :], t1[:, :, :],
                                                t2[:, :, :], OP.add)
                        tcc = scpool.tile([128, KH, BL], F32, tag="tcc",
                                          name="tcc")
                        nc.scalar.activation(tcc[:, :, :], cbuf[:, :, :],
                                             AF.Tanh)
                    else:
                        so = scpool.tile([128, KH, BL], F32, tag="so",
                                         name="so")
                        nc.scalar.activation(so[:, :, :], gs[2][:, :, :],
                                             AF.Sigmoid)
                        nc.vector.tensor_tensor(hbuf[:, nxt, :, :],
                                                so[:, :, :], tcc[:, :, :],
                                                OP.mult)
                nc.vector.tensor_tensor(hd[:, :, ds(cols, BL)],
                                        so[:, :, :], tcc[:, :, :], OP.mult)
                if hd2 is not None:
                    nc.vector.tensor_tensor(
                        hd2[:, :, ds((S - 1 - j) * BL, BL)],
                        so[:, :, :], tcc[:, :, :], OP.mult)

        # ---- layer 0: x.T features + projection, chunk-pipelined ---------
        xw0 = xwpool.tile([128, MT, SB], BF16, tag="xw", name="xw0")
        stx = st.enter_context(ExitStack())
        xtpool = stx.enter_context(tc.tile_pool(name="xtp", bufs=1))
        hlpool = stx.enter_context(tc.tile_pool(name="hl", bufs=5))
        sumpool = stx.enter_context(tc.tile_pool(name="sum", bufs=4))
        frpool = stx.enter_context(tc.tile_pool(name="frp", bufs=1))

        xt = xtpool.tile([128, K0, SB], BF16, tag="xt")
        hs_sbe = hs.rearrange("l b s e -> l s b e")
        rt_sum = {}

        def rowtile_dma(r):
            hl = []
            for layer in range(4):
                t = hlpool.tile([128, E], BF16, tag="hl")
                nc.sync.dma_start(
                    out=t[:, :],
                    in_=hs_sbe[layer, TPR * r:TPR * (r + 1), :, :])
                hl.append(t)
            s01 = sumpool.tile([128, E], F32, tag="sum")
            nc.vector.tensor_tensor(s01[:, :], hl[0][:, :], hl[1][:, :],
                                    OP.add)
            s23 = sumpool.tile([128, E], F32, tag="sum")
            nc.vector.tensor_tensor(s23[:, :], hl[2][:, :], hl[3][:, :],
                                    OP.add)
            ssum = sumpool.tile([128, E], F32, tag="sum")
            nc.vector.tensor_tensor(ssum[:, :], s01[:, :], s23[:, :], OP.add)
            rt_sum[r] = ssum

        def rowtile_tp(r):
            ssum = rt_sum.pop(r)
            for c in range(6):
                pt = pspool.tile([128, 128], F32, tag="tp")
                nc.tensor.transpose(pt[:, :], ssum[:, 128 * c:128 * (c + 1)],
                                    ident[:, :])
                nc.vector.tensor_scalar_mul(
                    xt[:, c, 128 * r:128 * (r + 1)], pt[:, :], 0.25)

        def proj_item(xw, ch, segs):
            """One m-tile, one 512-col chunk: weight DMAs + one accumulation
            chain + PSUM->SBUF copy. segs: list of (wih_dram, nk, rhs_of_k,
            m). Returns a thunk."""
            def emit():
                wms = []
                for (wih_dram, nk, _, m) in segs:
                    wm = wpool.tile([128, nk, 128], BF16, tag=f"wihm{nk}")
                    nc.sync.dma_start(out=wm[:, :, :], in_=wih_dram[m])
                    wms.append(wm)
                pp = psproj.tile([128, 512], F32, tag="proj")
                nks = sum(s[1] for s in segs)
                kk = 0
                for wm, (_, nk, rhs_of_k, m) in zip(wms, segs):
                    for k in range(nk):
                        nc.tensor.matmul(pp[:, :], wm[:, k, :],
                                         rhs_of_k(k, ch),
                                         start=(kk == 0), stop=(kk == nks - 1))
                        kk += 1
                m0 = segs[0][3]
                nc.vector.tensor_copy(xw[:, m0, 512 * ch:512 * (ch + 1)],
                                      pp[:, :])
            return emit

        def xt_rhs(k, ch):
            return xt[:, k, 512 * ch:512 * (ch + 1)]

        # feature rows (delta@p0, mask@p32, ones@p64) in xt[:, 6, :]
        nc.vector.memset(xt[:, 6, :], 0.0)
        nc.vector.memset(xt[64:65, 6, :], 1.0)
        frow = frpool.tile([1, SB], F32, tag="frow", name="frow_r")
        nc.sync.dma_start(out=frow[:, :], in_=rmask[:, :])
        nc.vector.tensor_copy(xt[32:33, 6, :], frow[:, :])
        frow2 = frpool.tile([1, SB], F32, tag="frow", name="frow_d")
        nc.sync.dma_start(out=frow2[:, :], in_=drow[:, :])
        nc.vector.tensor_copy(xt[0:1, 6, :], frow2[:, :])

        # chunk 0 up front; chunks 1-3 fill the L0 scan's stall gaps
        for r in range(RPC):
            rowtile_dma(r)
            rowtile_tp(r)
        for m in range(MT):
            proj_item(xw0, 0, [(w["wih0"], K0, xt_rhs, m)])()
        for c in range(1, NCH):
            for r in range(RPC * c, RPC * (c + 1)):
                filler.add(SPC * (c - 1), SPC * c - 8,
                           (lambda rr: lambda: rowtile_dma(rr))(r))
                filler.add(SPC * (c - 1), SPC * c - 4,
                           (lambda rr: lambda: rowtile_tp(rr))(r))
            for m in range(MT):
                filler.add(SPC * (c - 1), SPC * c,
                           proj_item(xw0, c, [(w["wih0"], K0, xt_rhs, m)]))

        scan_layer(whh0, xw0, hdst, hdst2)
        filler.drain()
        stx.close()
        latepool = st.enter_context(tc.tile_pool(name="late", bufs=1))

        # ---- exchange: AllReduce pair; hrecv = sum - own ----------------
        nc.sync.dma_start(out=hx_in.ap(), in_=hdst2[:, :, :])
        nc.gpsimd.collective_compute(
            "AllReduce", OP.add, replica_groups=RG,
            ins=[hx_in.ap().opt()], outs=[hx_out.ap().opt()])

        # L1 own-half chunk-0 projection overlaps the collective on the PE
        whh1 = wpool.tile([128, KH, G], BF16, tag="whh", name="whh1")
        nc.sync.dma_start(out=whh1[:, :, :], in_=w["whh1"][:, :, :])
        wo = wpool.tile([128, K1O, C], BF16, tag="wout")
        nc.sync.dma_start(out=wo[:, :, :], in_=w["wout"][:, :, :])

        def own_rhs(k, ch):
            if k < KH:
                return hdst[:, k, 512 * ch:512 * (ch + 1)]
            return ones_row[:, 512 * ch:512 * (ch + 1)]

        def rcv_rhs(k, ch):
            return hrecv[:, k, 512 * ch:512 * (ch + 1)]

        xw1 = xwpool.tile([128, MT, SB], BF16, tag="xw", name="xw1")
        for m in range(MT):
            proj_item(xw1, 0, [(w["wih1o"], K1O, own_rhs, m)])()

        hrecv = latepool.tile([128, KH, SB], BF16, tag="hrecv", name="hrecv")
        nc.sync.dma_start(out=hrecv[:, :, :], in_=hx_out.ap())
        nc.vector.tensor_tensor(hrecv[:, :, :], hrecv[:, :, :],
                                hdst2[:, :, :], OP.subtract)

        def rcv_acc_item(m, ch):
            def emit():
                wm = wpool.tile([128, K1R, 128], BF16, tag=f"wihm{K1R}")
                nc.sync.dma_start(out=wm[:, :, :], in_=w["wih1r"][m])
                pp = psproj.tile([128, 512], F32, tag="proj")
                for k in range(K1R):
                    nc.tensor.matmul(pp[:, :], wm[:, k, :], rcv_rhs(k, ch),
                                     start=(k == 0), stop=(k == K1R - 1))
                nc.vector.tensor_tensor(
                    xw1[:, m, 512 * ch:512 * (ch + 1)], pp[:, :],
                    xw1[:, m, 512 * ch:512 * (ch + 1)], OP.add)
            return emit

        for m in range(MT):
            rcv_acc_item(m, 0)()

        h1 = latepool.tile([128, KH, SB], BF16, tag="h1", name="h1")
        out_sbc = out.rearrange("b s c -> s b c")

        def outproj_item(ch):
            def emit():
                po = psproj.tile([C, 512], F32, tag="proj")
                for k in range(K1O):
                    if k < KH:
                        rhs = h1[:, k, 512 * ch:512 * (ch + 1)]
                    else:
                        rhs = ones_row[:, 512 * ch:512 * (ch + 1)]
                    nc.tensor.matmul(po[:, :], wo[:, k, :], rhs,
                                     start=(k == 0), stop=(k == K1O - 1))
                ost = scpool.tile([C, 512], F32, tag="ost")
                nc.vector.tensor_copy(ost[:, :], po[:, :])
                for cb in range(4):
                    pt = pspool.tile([128, C], F32, tag="tp")
                    nc.tensor.transpose(pt[:, :],
                                        ost[:, 128 * cb:128 * (cb + 1)],
                                        ident[0:C, 0:C])
                    onat = scpool.tile([128, C], F32, tag="onat")
                    nc.vector.tensor_copy(onat[:, :], pt[:, :])
                    gb = 4 * ch + cb
                    nc.sync.dma_start(
                        out=out_sbc[TPR * gb:TPR * (gb + 1), :, :],
                        in_=onat[:, :])
            return emit

        # L1 fillers: remaining xw1 chunks (own+recv 9-chains) trail one
        # chunk ahead of the scan; out-projection chunks trail completion.
        for c in range(1, NCH):
            for m in range(MT):
                filler.add(SPC * (c - 1), SPC * c,
                           proj_item(xw1, c,
                                     [(w["wih1o"], K1O, own_rhs, m),
                                      (w["wih1r"], K1R, rcv_rhs, m)]))
        for c in range(NCH - 1):
            filler.add(SPC * (c + 1), 10 ** 9, outproj_item(c))
        filler.add(10 ** 9, 10 ** 9, outproj_item(NCH - 1))

        scan_layer(whh1, xw1, h1, None)
        filler.drain()


def _get_nc():
    if "nc" not in _cache:
        _cache["nc"] = build_nc()
    return _cache["nc"]


def make_in_maps(inputs):
    wmaps = {d: _prep_weights(inputs, d) for d in ("f", "b")}
    return [_prep_core_inputs(inputs, wmaps, c) for c in range(NCORES)]


def kernel(**inputs):
    from concourse.bass_utils import run_bass_kernel_spmd

    in_maps = make_in_maps(inputs)
    nc = _get_nc()
    res = run_bass_kernel_spmd(nc, in_maps, core_ids=list(range(NCORES)))
    parts = [r["out"] for r in res.results]
    full = np.empty((B, S, C), np.float32)
    for p in range(NPAIR):
        full[BL * p:BL * (p + 1)] = parts[2 * p] + parts[2 * p + 1][:, ::-1]
    return full
